# revision 9
# baseline (speedup 1.0000x reference)
"""Trainium2 Bass kernel for CustomGATConv (dense masked GAT attention).

Strategy (8-core SPMD, row-sharded attention; v2 — instruction-count
minimal for this runtime, where every compute-engine instruction costs
~50-130us regardless of size while DMAs cost ~3us):

  - Each core owns 512 destination rows i (natural order, no rotation).
  - e_src/e_dst are computed exactly on the host (tiny GEMM x @ (W A)).
  - h = x @ W runs on PE in bf16 once per call (stage B).
  - Per rep (stage C), the [j=4096, h=8, i=512] logit tensor is built in
    SBUF by two bulk DVE adds using stride-0 broadcast access patterns
    (e_src broadcast over j-tiles, e_dst broadcast over i, additive
    -200 mask broadcast over heads) — no PSUM, no identity matmuls.
    Prelu and Exp run as two big ACT instructions per 2-j-tile group.
  - alpha @ h (with a ones-column for the softmax denominator) is 256
    bf16 matmuls accumulating into a single [65, 8, 512] PSUM tile that
    occupies all 8 banks across the whole j sweep.
  - Normalization: reciprocal of the denominator row, partition-doubling
    DMA broadcast, one DVE multiply, and a single strided store DMA.
"""

import re

import numpy as np
import ml_dtypes

import bass_rust as br
import concourse.bass as bass
import concourse.tile as tile
from concourse import mybir
from concourse.bass_utils import run_bass_kernel_spmd

N = 4096
IN = 256
H = 8
F = 64
NCORES = 8
R = N // NCORES          # 512 destination rows per core
JT = N // 128            # 32 j-tiles
JTG = 2                  # j-tiles per stage-C group
NEG = -200.0             # additive mask value
FP = mybir.dt.float32
BF = mybir.dt.bfloat16
F8 = mybir.dt.float8e4

USE_FP8 = False          # fp8e4 + DoubleRow (K=256) for alpha @ h
EXPB = -2.0              # exp bias: exp(z-2) keeps pp within fp8e4 range
NORM_MODE = "dma"


class _TileContext(tile.TileContext):
    """TileContext whose final drain splits its semaphore waits one per
    instruction — this walrus's CTRL_NO encoding only fits one sync wait."""

    def _drain_and_barrier(self, tick_clock, wait_clock):
        gc = tick_clock.global_clock
        vals = list(map(int, re.findall(r"\d+", repr(gc))))
        nonzero = [(i, t) for i, t in enumerate(vals) if t > 0]
        prev = br.VectorClock()
        partial = br.VectorClock()
        for i, t in nonzero:
            partial.require_at_least(i, t)
            inst = self.nc.sync.drain().ins
            wait_clock.add_sem_waits(
                inst,
                br.ScopedClock({None: partial.copy()}),
                br.ScopedClock({None: prev.copy()}),
            )
            prev = partial.copy()
        drain_inst = self.nc.sync.drain().ins
        wait_clock.add_sem_waits(
            drain_inst,
            br.ScopedClock({None: gc}),
            br.ScopedClock({None: prev.copy()}),
        )
        self.nc.all_engine_barrier()
        popped = self.nc._tile_sem_poison_stack.pop()
        assert popped is self._sem_poison
        self.nc.clear_and_free_semaphores(list(self.sems.allocated().values()))
        self.nc.all_engine_barrier()


def _split_excess_waits(nc, cap_compute=1, cap_nop=1):
    """This walrus encodes at most ~2 sync waits per compute instruction and
    1 per CTRL_NO (nop/drain).  Move excess waits onto injected same-engine
    nops placed immediately before the over-subscribed instruction."""
    n_split = 0
    for fn in nc.m.functions:
        for bb in fn.blocks:
            lst = bb.instructions
            i = 0
            while i < len(lst):
                inst = lst[i]
                si = inst.sync_info
                waits = list(si.on_wait) if si is not None else []
                is_ctrl = isinstance(inst, (mybir.InstNoOp, mybir.InstDrain))
                cap = cap_nop if is_ctrl else cap_compute
                if len(waits) > cap:
                    excess, keep = waits[:-cap], waits[-cap:]
                    for w in excess:
                        nop = mybir.InstNoOp(name=f"waitsplit-{nc.next_id()}")
                        nop.engine = inst.engine
                        nop.sync_info = br.SyncInfo(on_wait=[w], on_update=[])
                        lst.insert(i, nop)
                        i += 1
                        n_split += 1
                    inst.sync_info = br.SyncInfo(
                        on_wait=keep, on_update=list(si.on_update)
                    )
                i += 1
    return n_split


def _build_program(repeat=1):
    nc = bass.Bass("TRN2", target_bir_lowering=False, debug=False)
    ap = {}
    ap["xT"] = nc.dram_tensor("xT", [IN, N], BF, kind="ExternalInput").ap()
    ap["w"] = nc.dram_tensor("w", [IN, H * F], BF, kind="ExternalInput").ap()
    ap["maskadd"] = nc.dram_tensor("maskadd", [N, R], BF, kind="ExternalInput").ap()
    ap["esrcb"] = nc.dram_tensor("esrcb", [128, H, R], BF, kind="ExternalInput").ap()
    ap["edst"] = nc.dram_tensor("edst", [N, H], BF, kind="ExternalInput").ap()
    if NORM_MODE == "pe":
        ap["identf"] = nc.dram_tensor("identf", [128, 128], FP, kind="ExternalInput").ap()
    out_ap = nc.dram_tensor("out", [F, H * R], FP, kind="ExternalOutput").ap()

    with _TileContext(nc) as tc:
        _emit(tc, nc, ap, out_ap, repeat)
    _split_excess_waits(nc)
    return nc


def _emit(tc, nc, ap, out_ap, repeat):
    from contextlib import ExitStack

    Act = mybir.ActivationFunctionType
    with ExitStack() as ctx:
        singles = ctx.enter_context(tc.tile_pool(name="singles", bufs=1))

        # ---- persistent tiles ----
        mask_sb = singles.tile([128, JT, 1, R], BF)
        nc.sync.dma_start(
            mask_sb[:, :, 0, :], ap["maskadd"].rearrange("(t p) i -> p t i", p=128)
        )
        biasc_sb = singles.tile([128, 1], FP)
        nc.vector.memset(biasc_sb[:], EXPB)
        esrcb_sb = singles.tile([128, 1, H, R], BF)
        nc.sync.dma_start(esrcb_sb[:, 0, :, :], ap["esrcb"])
        edst_sb = singles.tile([128, JT, H, 1], BF)
        nc.sync.dma_start(
            edst_sb[:, :, :, 0], ap["edst"].rearrange("(t p) h -> p t h", p=128)
        )
        if USE_FP8:
            # lhsT pair stride must be a multiple of 16 -> pad 65 to 80
            haug2_sb = singles.tile([128, JT // 2, H, 2, 80], F8)
            for s in range(2):
                nc.vector.memset(
                    haug2_sb[:, :, :, s, F:F + 1].rearrange(
                        "p t h o -> p t (h o)"
                    ),
                    1.0,
                )
        else:
            haug_sb = singles.tile([128, JT, H, F + 1], BF)
            nc.vector.memset(haug_sb[:, :, :, F:F + 1], 1.0)
        if NORM_MODE == "pe":
            identf_sb = singles.tile([128, 128], FP)
            nc.sync.dma_start(identf_sb[:], ap["identf"])

        # ---- stage B: h = x @ W (bf16), packed into haug ----
        with tc.tile_pool(name="bigin", bufs=1) as bigin, \
             tc.tile_pool(name="hpsum", bufs=2, space="PSUM") as hpsum:
            xT_sb = bigin.tile([128, 2, N], BF)
            nc.sync.dma_start(xT_sb[:], ap["xT"].rearrange("(k p) n -> p k n", p=128))
            w_sb = bigin.tile([128, 2, H * F], BF)
            nc.sync.dma_start(w_sb[:], ap["w"].rearrange("(k p) f -> p k f", p=128))

            for mb in range(JT // 4):
                ph = hpsum.tile([128, 4, H * F], FP, tag="ph")
                for mq in range(4):
                    m = mb * 4 + mq
                    for k in range(2):
                        nc.tensor.matmul(
                            ph[:, mq, :],
                            lhsT=xT_sb[:, k, m * 128:(m + 1) * 128],
                            rhs=w_sb[:, k, :],
                            start=(k == 0),
                            stop=(k == 1),
                        )
                if USE_FP8:
                    for mq in range(4):
                        m = mb * 4 + mq
                        nc.vector.tensor_copy(
                            out=haug2_sb[:, m // 2, :, m % 2, 0:F],
                            in_=ph[:, mq, :].rearrange("p (h f) -> p h f", h=H),
                        )
                else:
                    nc.vector.tensor_copy(
                        out=haug_sb[:, mb * 4:(mb + 1) * 4, :, 0:F],
                        in_=ph[:].rearrange("p q (h f) -> p q h f", h=H),
                    )

        # ---- stage C: masked softmax + alpha @ h ----
        opool = ctx.enter_context(tc.tile_pool(name="opool", bufs=1, space="PSUM"))
        zpool = ctx.enter_context(tc.tile_pool(name="zpool", bufs=2))
        ppool = ctx.enter_context(tc.tile_pool(name="ppool", bufs=2))
        npool = ctx.enter_context(tc.tile_pool(name="npool", bufs=2))

        bshape = [128, H, 2, R]
        for _rep in range(repeat):
            pout = opool.tile([F + 1, H, R], FP, tag="pout")
            if USE_FP8:
                for tp in range(JT // 2):
                    t0 = tp * 2
                    zt = zpool.tile(bshape, BF, tag="zt")
                    nc.vector.tensor_tensor(
                        out=zt[:],
                        in0=esrcb_sb[:].rearrange(
                            "p o h i -> p h o i"
                        ).broadcast_to(bshape),
                        in1=edst_sb[:, t0:t0 + 2, :, :].rearrange(
                            "p t h o -> p h t o"
                        ).broadcast_to(bshape),
                        op=mybir.AluOpType.add,
                    )
                    zp = zpool.tile(bshape, BF, tag="zp")
                    nc.vector.tensor_tensor(
                        out=zp[:],
                        in0=zt[:],
                        in1=mask_sb[:, t0:t0 + 2, :, :].rearrange(
                            "p t o i -> p o t i"
                        ).broadcast_to(bshape),
                        op=mybir.AluOpType.add,
                    )
                    zpf = zp[:].rearrange("p h s i -> p (h s i)")
                    ztf = zt[:].rearrange("p h s i -> p (h s i)")
                    nc.scalar.activation(out=ztf, in_=zpf, func=Act.Prelu, alpha=0.2)
                    pp = ppool.tile(bshape, F8, tag="pp")
                    ppf = pp[:].rearrange("p h s i -> p (h s i)")
                    nc.scalar.activation(out=ppf, in_=ztf, func=Act.Exp, bias=biasc_sb[:])
                    for h in range(H):
                        nc.tensor.matmul(
                            pout[:, h, :],
                            lhsT=haug2_sb[:, tp, h, :, 0:F + 1],
                            rhs=pp[:, h, :, :],
                            start=(tp == 0),
                            stop=(tp == JT // 2 - 1),
                            perf_mode=mybir.MatmulPerfMode.DoubleRow,
                            skip_group_check=True,
                        )
            else:
                for g in range(JT // JTG):
                    t0 = g * JTG
                    vshape = [128, JTG, H, R]
                    zt = zpool.tile(vshape, BF, tag="zt")
                    nc.vector.tensor_tensor(
                        out=zt[:],
                        in0=esrcb_sb[:].broadcast_to(vshape),
                        in1=edst_sb[:, t0:t0 + JTG, :, :].broadcast_to(vshape),
                        op=mybir.AluOpType.add,
                    )
                    zp = ppool.tile(vshape, BF, tag="zp")
                    nc.vector.tensor_tensor(
                        out=zp[:],
                        in0=zt[:],
                        in1=mask_sb[:, t0:t0 + JTG, :, :].broadcast_to(vshape),
                        op=mybir.AluOpType.add,
                    )
                    zpf = zp[:].rearrange("p t h i -> p (t h i)")
                    ztf = zt[:].rearrange("p t h i -> p (t h i)")
                    nc.scalar.activation(out=ztf, in_=zpf, func=Act.Prelu, alpha=0.2)
                    nc.scalar.activation(out=zpf, in_=ztf, func=Act.Exp)
                    for tl in range(JTG):
                        jt = t0 + tl
                        for h in range(H):
                            nc.tensor.matmul(
                                pout[:, h, :],
                                lhsT=haug_sb[:, jt, h, :],
                                rhs=zp[:, tl, h, :],
                                start=(jt == 0),
                                stop=(jt == JT - 1),
                                skip_group_check=True,
                            )

            # ---- normalize + store ----
            rcp = npool.tile([64, H, R], FP, tag="rcp")
            nc.vector.reciprocal(rcp[0:1, :, :], pout[F:F + 1, :, :])
            for d in (1, 2, 4, 8, 16, 32):
                nc.gpsimd.dma_start(out=rcp[d:2 * d], in_=rcp[0:d])
            osb = npool.tile([64, H, R], FP, tag="osb")
            nc.vector.tensor_tensor(
                out=osb[:],
                in0=pout[0:F, :, :],
                in1=rcp[:],
                op=mybir.AluOpType.mult,
            )
            nc.sync.dma_start(
                out_ap, osb[:].rearrange("f h i -> f (h i)")
            )


def _host_prep(x, edge_index, W, a):
    x = np.asarray(x, np.float32)
    W = np.asarray(W, np.float32)
    a = np.asarray(a, np.float32)
    src = np.asarray(edge_index[0]).astype(np.int64)
    dst = np.asarray(edge_index[1]).astype(np.int64)

    # exact e_src / e_dst on host: e = x @ (W A)
    A = np.zeros((H * F, 2 * H), np.float32)
    for h in range(H):
        A[h * F:(h + 1) * F, h] = a[h, :F]
        A[h * F:(h + 1) * F, 8 + h] = a[h, F:]
    ea = x @ (W @ A)                       # [N, 16]
    esrc = ea[:, :H]                       # [N, 8]
    edst = np.ascontiguousarray(ea[:, H:]).astype(ml_dtypes.bfloat16)  # [N, 8]

    # additive mask, full matrix [j, i]: 0 iff edge (src=i, dst=j) or i==j
    mfull = np.full((N, N), NEG, ml_dtypes.bfloat16)
    mfull[dst, src] = 0.0
    idx = np.arange(N)
    mfull[idx, idx] = 0.0

    xTb = np.ascontiguousarray(x.T.astype(ml_dtypes.bfloat16))
    wb = W.astype(ml_dtypes.bfloat16)

    in_maps = []
    for c in range(NCORES):
        sl = slice(c * R, (c + 1) * R)
        esrcb = np.ascontiguousarray(
            np.broadcast_to(
                esrc[sl].T.astype(ml_dtypes.bfloat16)[None], (128, H, R)
            )
        )
        m = {
            "xT": xTb,
            "w": wb,
            "maskadd": np.ascontiguousarray(mfull[:, sl]),
            "esrcb": esrcb,
            "edst": edst,
        }
        if NORM_MODE == "pe":
            m["identf"] = np.eye(128, dtype=np.float32)
        in_maps.append(m)
    return in_maps


_CACHED = {}


def _get_program(repeat=1):
    if repeat not in _CACHED:
        _CACHED[repeat] = _build_program(repeat)
    return _CACHED[repeat]


def kernel(x, edge_index, W, a, _repeat=1):
    nc = _get_program(_repeat)
    in_maps = _host_prep(x, edge_index, W, a)
    res = run_bass_kernel_spmd(nc, in_maps, core_ids=list(range(NCORES)))
    # device output is [64 f, 8 h, 512 i] per core; transpose to [i, (h f)]
    out = np.empty((N, H * F), np.float32)
    for c in range(NCORES):
        arr = res.results[c]["out"].reshape(F, H, R)
        out[c * R:(c + 1) * R] = (
            np.transpose(arr, (2, 1, 0)).reshape(R, H * F)
        )
    return out
